# revision 4
# baseline (speedup 1.0000x reference)
"""Multi-head attention (B=2, S=2048, E=1024, H=16) on 8 Trainium2 NeuronCores.

Sharding v2 — query-block data parallel with an early K/V all-gather:
core c owns batch b=c//4 and query tokens q0=(c%4)*512 .. q0+512. Each core
projects Q/K/V only for its OWN 512 tokens (1/4 of the K/V projection work of
the zero-collective scheme), then the 4 cores sharing a batch all-gather K
(dh-major) and V (token-major natural layout) through internal DRAM, and each
core runs attention over all 2048 keys for all 16 heads plus the output
projection for its 512 tokens. The all-gather is split in two (head-blocks
0-3, then 4-7) so the first collective overlaps the second half of the
projections and attention on head-blocks 0-3 overlaps the second collective.

Softmax-exp is the scalar-engine bottleneck (16.8M exps/core), so a subset of
key tiles per head-block is computed on the vector engine instead using a
bf16 Schraudolph fast-exp (int16 bit-trick, ~3% per-element, which washes out
in the softmax-weighted average), keeping ACT, DVE and PE all busy.

Bias algebra: the K bias is softmax-invariant (q.bk is constant across keys)
and the V bias is a constant vector added to every context row, so both are
dropped on-device; the V bias is folded on the host into the output bias
(bo2 = bo + bv @ Wo). Only the Q bias is applied on-chip.
"""

import sys

if "/opt/trn_rl_repo" not in sys.path:
    sys.path.insert(0, "/opt/trn_rl_repo")

import numpy as np

B, S, E, H, DH = 2, 2048, 1024, 16, 64
NCORES = 8
NQ = 512           # query tokens per core (also its owned K/V token block)
HB = 8             # head-blocks of 128 v-dims (2 heads) each
NJ = 16            # 128-wide key tiles over the full 2048 keys
GROUPS = ((0, 1, 2, 3), (4, 5, 6, 7))

# all-gather buffer layout (elements, bf16), per group g of 4 head-blocks:
#   K part: [4 hbl][128 dh][512 keys]    at hbl*65536
#   V part: [4 hbl][4 jj][128 tok][128c] at VOFF + hbl*65536 + jj*16384
AGIN = 4 * 128 * 512 * 2
VOFF = 4 * 128 * 512

# vector-engine fast-exp (int16 bits -> bf16): e^(0.125*s) for raw score s
EXP_SA = 23.083120654223414
EXP_SB = 16250.65
# key tiles whose exp runs on the vector engine (rest on scalar engine)
DVE_J = (2, 4, 7, 9, 12, 14)

_CACHE = {}


def _build():
    from contextlib import ExitStack

    import concourse.bacc as bacc
    import concourse.mybir as mybir
    import concourse.tile as tile
    from concourse.masks import make_identity

    f32 = mybir.dt.float32
    f32r = mybir.dt.float32r
    bf16 = mybir.dt.bfloat16
    i16 = mybir.dt.int16

    nc = bacc.Bacc("TRN2", num_devices=NCORES)

    xt_d = nc.declare_dram_parameter("xt", [E, NQ], bf16, isOutput=False)
    wq_d = nc.declare_dram_parameter("wq", [E, E], bf16, isOutput=False)
    wk_d = nc.declare_dram_parameter("wk", [E, E], bf16, isOutput=False)
    wv_d = nc.declare_dram_parameter("wv", [E, E], bf16, isOutput=False)
    wo_d = nc.declare_dram_parameter("wo", [E, E], bf16, isOutput=False)
    bq_d = nc.declare_dram_parameter("bq", [E], f32, isOutput=False)
    bo_d = nc.declare_dram_parameter("bo", [E], f32, isOutput=False)
    out_d = nc.declare_dram_parameter("out", [NQ, E], f32, isOutput=True)

    ag_in = [nc.dram_tensor(f"agin{g}", [AGIN], bf16) for g in range(2)]
    ag_out = [nc.dram_tensor(f"agout{g}", [4, AGIN], bf16) for g in range(2)]

    with tile.TileContext(nc) as tc, ExitStack() as ctx:
        singles = ctx.enter_context(tc.tile_pool(name="singles", bufs=1))

        ident = singles.tile([128, 128], f32)
        make_identity(nc, ident)

        # constants for softmax-denominator broadcast + head-pair stacking
        cb64 = singles.tile([128, 64], f32r, tag="cb64")  # all 1/64
        nc.vector.tensor_scalar(
            out=cb64, in0=ident[:, 0:64], scalar1=0.0, scalar2=1.0 / 64.0,
            op0=mybir.AluOpType.mult, op1=mybir.AluOpType.add,
        )
        # stk[:, h, :]: [64, 128] with I64 in cols h*64..h*64+64
        stk = singles.tile([64, 2, 128], bf16, tag="stk")
        for h in range(2):
            nc.vector.tensor_scalar(
                out=stk[:, h, :], in0=ident[0:64, :],
                scalar1=0.0, scalar2=0.0,
                op0=mybir.AluOpType.mult, op1=mybir.AluOpType.add,
            )
        nc.vector.tensor_copy(out=stk[:, 0, 0:64], in_=ident[0:64, 0:64])
        nc.vector.tensor_copy(out=stk[:, 1, 64:128], in_=ident[0:64, 0:64])

        # Q bias: [128, 8] (col = head-block)
        bq_sb = singles.tile([128, 8], f32, tag="bq")
        nc.sync.dma_start(
            out=bq_sb, in_=bq_d.ap().rearrange("(o p) -> p o", p=128)
        )

        # persistent activations
        xT = singles.tile([128, 8, NQ], bf16, tag="xT")        # own x^T, 1 MB
        qT = singles.tile([128, 8, NQ], bf16, tag="qT")        # own q^T, 1 MB
        ctxT_sb = singles.tile([128, HB, NQ], bf16, tag="ctxT")

        for half in range(2):
            osl = slice(half * 4, (half + 1) * 4)
            nc.sync.dma_start(
                out=xT[:, osl, :],
                in_=xt_d.ap()[half * 512:(half + 1) * 512, :].rearrange(
                    "(o p) t -> p o t", p=128
                ),
            )

        # wv resident for the natural-layout V projection; wo for phase C.
        wbig = ctx.enter_context(tc.tile_pool(name="wbig", bufs=1))
        wv_sb = wbig.tile([128, 8, E], bf16, tag="wv")
        wo_sb = wbig.tile([128, 8, E], bf16, tag="wo")
        for w_s, w_d in ((wv_sb, wv_d), (wo_sb, wo_d)):
            for eh in range(2):
                nc.gpsimd.dma_start(
                    out=w_s[:, :, eh * 512:(eh + 1) * 512],
                    in_=w_d.ap()[:, eh * 512:(eh + 1) * 512].rearrange(
                        "(o p) e -> p o e", p=128
                    ),
                )
        bo_bc = wbig.tile([128, E], f32, tag="bo")
        nc.gpsimd.dma_start(
            out=bo_bc, in_=bo_d.ap().unsqueeze(0).broadcast_to([128, E])
        )

        # --- phase P: project own-token K/V (+ all-gather) then Q ---
        with (
            tc.tile_pool(name="wkq", bufs=2) as wkq,
            tc.tile_pool(name="stage", bufs=4) as stage,
            tc.tile_pool(name="projps", bufs=3, space="PSUM") as projps,
        ):
            for g in range(2):
                for hbl in range(4):
                    hb = g * 4 + hbl
                    csl = slice(hb * 128, (hb + 1) * 128)
                    wk_s = wkq.tile([128, 8, 128], bf16, tag="wk")
                    nc.sync.dma_start(
                        out=wk_s,
                        in_=wk_d.ap()[:, csl].rearrange("(o p) c -> p o c", p=128),
                    )
                    ps = projps.tile([128, NQ], f32, tag="proj")
                    for ec in range(8):
                        nc.tensor.matmul(
                            ps, wk_s[:, ec, :], xT[:, ec, :],
                            start=(ec == 0), stop=(ec == 7),
                        )
                    kst = stage.tile([128, NQ], bf16, tag="kst")
                    nc.scalar.copy(out=kst, in_=ps)
                    nc.scalar.dma_start(
                        out=ag_in[g].ap()[hbl * 65536:(hbl + 1) * 65536].rearrange(
                            "(p t) -> p t", p=128
                        ),
                        in_=kst,
                    )
                for jj in range(4):
                    jsl = slice(jj * 128, (jj + 1) * 128)
                    ps = projps.tile([128, NQ], f32, tag="proj")
                    for ec in range(8):
                        nc.tensor.matmul(
                            ps, xT[:, ec, jsl],
                            wv_sb[:, ec, g * 512:(g + 1) * 512],
                            start=(ec == 0), stop=(ec == 7),
                        )
                    vst = stage.tile([128, NQ], bf16, tag="vst")
                    nc.scalar.copy(out=vst, in_=ps)
                    for hbl in range(4):
                        off = VOFF + hbl * 65536 + jj * 16384
                        nc.scalar.dma_start(
                            out=ag_in[g].ap()[off:off + 16384].rearrange(
                                "(p c) -> p c", p=128
                            ),
                            in_=vst[:, hbl * 128:(hbl + 1) * 128],
                        )
                nc.gpsimd.collective_compute(
                    "AllGather",
                    mybir.AluOpType.bypass,
                    replica_groups=[list(g_) for g_ in GROUPS],
                    ins=[ag_in[g].ap().opt()],
                    outs=[ag_out[g].ap().opt()],
                )

            for hb in range(HB):
                csl = slice(hb * 128, (hb + 1) * 128)
                wq_s = wkq.tile([128, 8, 128], bf16, tag="wq")
                nc.sync.dma_start(
                    out=wq_s,
                    in_=wq_d.ap()[:, csl].rearrange("(o p) c -> p o c", p=128),
                )
                ps = projps.tile([128, NQ], f32, tag="proj")
                for ec in range(8):
                    nc.tensor.matmul(
                        ps, wq_s[:, ec, :], xT[:, ec, :],
                        start=(ec == 0), stop=(ec == 7),
                    )
                nc.vector.tensor_scalar_add(
                    out=qT[:, hb, :], in0=ps, scalar1=bq_sb[:, hb:hb + 1]
                )

        # --- phase A: attention per head-block over gathered K/V ---
        with (
            tc.tile_pool(name="kpool", bufs=2) as kpool,
            tc.tile_pool(name="vpool", bufs=2) as vpool,
            tc.tile_pool(name="expool", bufs=3) as expool,
            tc.tile_pool(name="dvp", bufs=2) as dvp,
            tc.tile_pool(name="stps", bufs=3, space="PSUM") as stps,
            tc.tile_pool(name="ctxps", bufs=2, space="PSUM") as ctxps,
        ):
            for hb in range(HB):
                g, hbl = hb // 4, hb % 4

                kT = kpool.tile([128, S], bf16, tag="kT")
                for r in range(4):
                    nc.sync.dma_start(
                        out=kT[:, r * 512:(r + 1) * 512],
                        in_=ag_out[g].ap()[
                            r, hbl * 65536:(hbl + 1) * 65536
                        ].rearrange("(p t) -> p t", p=128),
                    )
                von = vpool.tile([128, NJ, 2, 128], bf16, tag="von")
                # ones planes (cols 64:128) for the PE row-sum trick; gpsimd
                # is otherwise idle. memset can't hit f32-for-PE tiles, so
                # fill via x*0+1 from xT (always initialized by now).
                nc.gpsimd.tensor_scalar(
                    out=von[:, :, :, 64:128],
                    in0=xT[:, 0:4, :].rearrange(
                        "p o (a h c) -> p (o a) h c", a=4, h=2, c=64
                    ),
                    scalar1=0.0, scalar2=1.0,
                    op0=mybir.AluOpType.mult, op1=mybir.AluOpType.add,
                )
                for r in range(4):
                    for jj in range(4):
                        off = VOFF + hbl * 65536 + jj * 16384
                        nc.sync.dma_start(
                            out=von[:, r * 4 + jj, :, 0:64],
                            in_=ag_out[g].ap()[r, off:off + 16384].rearrange(
                                "(p c) -> p c", p=128
                            ),
                        )

                # attention, h0/h1 interleaved per key tile: the two score
                # matmuls use disjoint PE row-halves (tile_position from the
                # 64-partition base). One exp op covers both heads' scores;
                # exp runs on ACT except for DVE_J tiles (fast-exp on DVE).
                ctx0 = ctxps.tile([128, NQ], f32, tag="ctx")
                ctx1 = ctxps.tile([128, NQ], f32, tag="ctx")
                ctx_ps = [ctx0, ctx1]
                for j in range(NJ):
                    jsl = slice(j * 128, (j + 1) * 128)
                    st2 = stps.tile([128, 2, NQ], f32, tag="st")
                    for h in range(2):
                        hr = slice(64 * h, 64 * h + 64)
                        nc.tensor.matmul(
                            st2[:, h, :], kT[hr, jsl], qT[hr, hb, :],
                            start=True, stop=True,
                        )
                    if j in DVE_J:
                        exi = expool.tile([128, 2 * NQ], i16, tag="exi")
                        nc.vector.tensor_scalar(
                            out=exi,
                            in0=st2.rearrange("p a b -> p (a b)"),
                            scalar1=EXP_SA, scalar2=EXP_SB,
                            op0=mybir.AluOpType.mult, op1=mybir.AluOpType.add,
                        )
                        exv = exi.bitcast(bf16)
                    else:
                        ex2 = expool.tile([128, 2 * NQ], bf16, tag="ex")
                        nc.scalar.activation(
                            out=ex2,
                            in_=st2.rearrange("p a b -> p (a b)"),
                            func=mybir.ActivationFunctionType.Exp, scale=0.125,
                        )
                        exv = ex2
                    for h in range(2):
                        nc.tensor.matmul(
                            ctx_ps[h], von[:, j, h, :],
                            exv[:, h * NQ:(h + 1) * NQ],
                            start=(j == 0), stop=(j == NJ - 1),
                        )

                # normalize each head at partition offset 0 (exp row-sums are
                # on partitions 64:128 -> PE-broadcast down via cb64), then
                # stack the normalized pair into ctxT_sb[:, hb, :] on the PE.
                cn = [None, None]
                for h in range(2):
                    l_sb = dvp.tile([128, NQ], f32r, tag=f"lsb{h}")
                    nc.vector.tensor_copy(
                        out=l_sb[64:128, :], in_=ctx_ps[h][64:128, :]
                    )
                    lr = stps.tile([64, NQ], f32, tag="st")
                    nc.tensor.matmul(
                        lr, cb64[64:128, :], l_sb[64:128, :], start=True, stop=True
                    )
                    recip = dvp.tile([64, NQ], f32, tag=f"recip{h}")
                    nc.vector.reciprocal_approx_fast(out=recip, in_=lr)
                    cnh = dvp.tile([64, NQ], bf16, tag=f"cn{h}")
                    nc.vector.tensor_mul(
                        out=cnh, in0=ctx_ps[h][0:64, :], in1=recip
                    )
                    cn[h] = cnh
                pair_ps = stps.tile([128, NQ], f32, tag="st")
                nc.tensor.matmul(pair_ps, stk[:, 0, :], cn[0], start=True, stop=False)
                nc.tensor.matmul(pair_ps, stk[:, 1, :], cn[1], start=False, stop=True)
                nc.scalar.copy(out=ctxT_sb[:, hb, :], in_=pair_ps)

        # --- phase C: output projection for own 512-token slice ---
        with (
            tc.tile_pool(name="ph4", bufs=3) as ph4,
            tc.tile_pool(name="ph4ps", bufs=2, space="PSUM") as ph4ps,
        ):
            for tt in range(4):
                for eh in range(2):
                    esl = slice(eh * 512, (eh + 1) * 512)
                    ps_o = ph4ps.tile([128, 512], f32, tag="o")
                    for hb in range(HB):
                        nc.tensor.matmul(
                            ps_o,
                            ctxT_sb[:, hb, tt * 128:(tt + 1) * 128],
                            wo_sb[:, hb, esl],
                            start=(hb == 0), stop=(hb == HB - 1),
                        )
                    o_sb = ph4.tile([128, 512], f32, tag="osb")
                    nc.vector.tensor_add(out=o_sb, in0=ps_o, in1=bo_bc[:, esl])
                    nc.sync.dma_start(
                        out=out_d.ap()[tt * 128:(tt + 1) * 128, esl], in_=o_sb
                    )

    nc.finalize()
    return nc


def _get_nc():
    if "nc" not in _CACHE:
        _CACHE["nc"] = _build()
    return _CACHE["nc"]


def kernel(x, Wq, bq, Wk, bk, Wv, bv, Wo, bo, **_ignored):
    import ml_dtypes
    from concourse.bass_utils import run_bass_kernel_spmd

    bf = ml_dtypes.bfloat16
    x = np.asarray(x, dtype=np.float32)
    Wqb = np.ascontiguousarray(np.asarray(Wq, dtype=np.float32).astype(bf))
    Wkb = np.ascontiguousarray(np.asarray(Wk, dtype=np.float32).astype(bf))
    Wvb = np.ascontiguousarray(np.asarray(Wv, dtype=np.float32).astype(bf))
    Wob = np.ascontiguousarray(np.asarray(Wo, dtype=np.float32).astype(bf))
    bq = np.ascontiguousarray(np.asarray(bq, dtype=np.float32))
    # K bias is softmax-invariant; V bias folds into the output bias exactly.
    bo2 = np.ascontiguousarray(
        (
            np.asarray(bo, dtype=np.float64)
            + np.asarray(bv, dtype=np.float64) @ np.asarray(Wo, dtype=np.float64)
        ).astype(np.float32)
    )

    in_maps = []
    for c in range(NCORES):
        b, q0 = c // 4, (c % 4) * NQ
        xt = np.ascontiguousarray(x[b, q0:q0 + NQ].T.astype(bf))
        in_maps.append(
            {
                "xt": xt,
                "wq": Wqb, "wk": Wkb, "wv": Wvb, "wo": Wob,
                "bq": bq, "bo": bo2,
            }
        )

    nc = _get_nc()
    import os

    trace = bool(int(os.environ.get("MHA_TRACE", "0")))
    res = run_bass_kernel_spmd(
        nc, in_maps, core_ids=list(range(NCORES)), trace=trace
    )
    if trace:
        _CACHE["last_results"] = res
    _CACHE["res"] = res
    out = np.empty((B, S, E), dtype=np.float32)
    for c in range(NCORES):
        b, q0 = c // 4, (c % 4) * NQ
        out[b, q0:q0 + NQ] = res.results[c]["out"]
    return out


# revision 7
# speedup vs baseline: 1.0639x; 1.0639x over previous
"""Multi-head attention (B=2, S=2048, E=1024, H=16) on 8 Trainium2 NeuronCores.

Sharding v3 — query-block data parallel with an early K/V all-gather:
core c owns batch b=c//4 and query tokens q0=(c%4)*512. Each core projects
Q/K/V only for its OWN 512 tokens (1/4 of the K/V projection work of the
zero-collective scheme), the 4 cores sharing a batch all-gather K and V
through internal DRAM (one collective per head-block half so the first
overlaps the second half of the projections), and each core runs attention
over all 2048 keys for all 16 heads plus the output projection for its own
512 tokens.

Collective payload engineering:
 - K and Q are quantized to fp8-e3m4 (the score matmul runs fp8xfp8 at the
   same 1 cycle/row as bf16; the ~1.5% element error on scores washes out in
   the softmax average). That halves the K bytes through the collective.
 - V ships in "plane" layout [h0_v(64) | ones(64) | h1_v(64)] per key tile,
   so each attention-side load is one fully contiguous 128x768B DMA and the
   ones columns for the PE row-sum (softmax denominator) trick arrive
   in-band: ctx lhsT for h0 is cols 0:128, for h1 cols 64:192 (denominators
   land mirrored on partitions 0:64 for h1; normalization accounts for it).

Softmax-exp is the scalar-engine bottleneck (16.8M exps/core), so DVE_J key
tiles per head-block run on the vector engine instead via a bf16 Schraudolph
fast-exp (int16 bit-trick, ~3% per-element, washes out in the softmax
average), keeping ACT, DVE and PE all busy (~11us/head-block each).

Bias algebra: the K bias is softmax-invariant (q.bk is constant across keys)
and the V bias is a constant vector added to every context row, so both are
dropped on-device; the V bias is folded on the host into the output bias
(bo2 = bo + bv @ Wo). Only the Q bias is applied on-chip.
"""

import sys

if "/opt/trn_rl_repo" not in sys.path:
    sys.path.insert(0, "/opt/trn_rl_repo")

import numpy as np

B, S, E, H, DH = 2, 2048, 1024, 16, 64
NCORES = 8
NQ = 512           # query tokens per core (also its owned K/V token block)
HB = 8             # head-blocks of 128 v-dims (2 heads) each
NJ = 16            # 128-wide key tiles over the full 2048 keys
GROUPS = ((0, 1, 2, 3), (4, 5, 6, 7))

# all-gather buffer layout in fp8 elements (= bytes), per group g:
#   K part: [4 hbl][128 dh][512 keys] fp8            at hbl*65536
#   V part: [4 hbl][4 jj][128 tok][384 (=192 bf16)]  at VOFF + hbl*196608 + jj*49152
VOFF = 4 * 128 * 512
AGIN = VOFF + 4 * 4 * 128 * 384

# vector-engine fast-exp (int16 bits -> bf16): e^(0.125*s) for raw score s
EXP_SA = 23.083120654223414
EXP_SB = 16250.65
# key tiles whose exp runs on the vector engine (rest on scalar engine)
DVE_J = (2, 4, 7, 9, 12, 14)

_CACHE = {}


def _build():
    from contextlib import ExitStack

    import concourse.bacc as bacc
    import concourse.mybir as mybir
    import concourse.tile as tile
    from concourse.masks import make_identity

    f32 = mybir.dt.float32
    f32r = mybir.dt.float32r
    bf16 = mybir.dt.bfloat16
    f8 = mybir.dt.float8e3
    i16 = mybir.dt.int16

    nc = bacc.Bacc("TRN2", num_devices=NCORES)

    xt_d = nc.declare_dram_parameter("xt", [E, NQ], bf16, isOutput=False)
    wq_d = nc.declare_dram_parameter("wq", [E, E], bf16, isOutput=False)
    wk_d = nc.declare_dram_parameter("wk", [E, E], bf16, isOutput=False)
    wv_d = nc.declare_dram_parameter("wv", [E, E], bf16, isOutput=False)
    wo_d = nc.declare_dram_parameter("wo", [E, E], bf16, isOutput=False)
    bq_d = nc.declare_dram_parameter("bq", [E], f32, isOutput=False)
    bo_d = nc.declare_dram_parameter("bo", [E], f32, isOutput=False)
    out_d = nc.declare_dram_parameter("out", [NQ, E], f32, isOutput=True)

    ag_in = [nc.dram_tensor(f"agin{g}", [AGIN], f8) for g in range(2)]
    ag_out = [nc.dram_tensor(f"agout{g}", [4, AGIN], f8) for g in range(2)]

    with tile.TileContext(nc) as tc, ExitStack() as ctx:
        singles = ctx.enter_context(tc.tile_pool(name="singles", bufs=1))

        # wk first on the sync queue: the very first matmul needs only
        # wk chunk 0 + xT chunk 0, so keep the gating transfers small.
        wk_sb = singles.tile([128, 8, E], bf16, tag="wk")
        xT = singles.tile([128, 8, NQ], bf16, tag="xT")
        for oc in range(4):
            nc.sync.dma_start(
                out=wk_sb[:, 2 * oc:2 * oc + 2, :],
                in_=wk_d.ap()[oc * 256:(oc + 1) * 256, :].rearrange(
                    "(o p) c -> p o c", p=128
                ),
            )
            for e2 in range(2):
                o = 2 * oc + e2
                nc.sync.dma_start(
                    out=xT[:, o, :],
                    in_=xt_d.ap()[o * 128:(o + 1) * 128, :].rearrange(
                        "(o p) t -> p o t", p=128
                    ),
                )
        wq_sb = singles.tile([128, 8, E], bf16, tag="wq")
        for oc in range(4):
            nc.sync.dma_start(
                out=wq_sb[:, 2 * oc:2 * oc + 2, :],
                in_=wq_d.ap()[oc * 256:(oc + 1) * 256, :].rearrange(
                    "(o p) c -> p o c", p=128
                ),
            )

        ident = singles.tile([128, 128], f32)
        make_identity(nc, ident)

        # cbX: block-antidiagonal 1/64 — one matmul averages the 64 identical
        # denominator copies of each head AND swaps them to the opposite
        # partition half (h0 denominators live on parts 64:128, its
        # numerators on 0:64; h1 is mirrored).
        cbX = singles.tile([128, 128], f32r, tag="cbX")
        nc.vector.tensor_scalar(
            out=cbX, in0=ident, scalar1=0.0, scalar2=0.0,
            op0=mybir.AluOpType.mult, op1=mybir.AluOpType.add,
        )
        for rs, cs in ((slice(64, 128), slice(0, 64)), (slice(0, 64), slice(64, 128))):
            nc.vector.tensor_scalar(
                out=cbX[rs, cs], in0=ident[rs, cs], scalar1=0.0, scalar2=1.0 / 64.0,
                op0=mybir.AluOpType.mult, op1=mybir.AluOpType.add,
            )
        # stk2: pair-stacking weights. h0: I64 at parts 0:64 -> cols 0:64;
        # h1: I64 at parts 64:128 -> cols 64:128 (h1 numerators live on the
        # upper partition half because its von plane is [ones | v]).
        stk2 = singles.tile([128, 2, 128], bf16, tag="stk2")
        for h in range(2):
            nc.vector.tensor_scalar(
                out=stk2[:, h, :], in0=ident,
                scalar1=0.0, scalar2=0.0,
                op0=mybir.AluOpType.mult, op1=mybir.AluOpType.add,
            )
        nc.vector.tensor_copy(out=stk2[0:64, 0, 0:64], in_=ident[0:64, 0:64])
        nc.vector.tensor_copy(out=stk2[64:128, 1, 64:128], in_=ident[64:128, 64:128])

        # Q bias: [128, 8] (col = head-block)
        bq_sb = singles.tile([128, 8], f32, tag="bq")
        nc.sync.dma_start(
            out=bq_sb, in_=bq_d.ap().rearrange("(o p) -> p o", p=128)
        )

        qT = singles.tile([128, 8, NQ], f8, tag="qT")       # own q^T, fp8
        ctxT_sb = singles.tile([128, HB, NQ], bf16, tag="ctxT")

        # wv resident for the natural-layout V projection; wo for phase C.
        wbig = ctx.enter_context(tc.tile_pool(name="wbig", bufs=1))
        wv_sb = wbig.tile([128, 8, E], bf16, tag="wv")
        wo_sb = wbig.tile([128, 8, E], bf16, tag="wo")
        for w_s, w_d in ((wv_sb, wv_d), (wo_sb, wo_d)):
            for eh in range(2):
                nc.gpsimd.dma_start(
                    out=w_s[:, :, eh * 512:(eh + 1) * 512],
                    in_=w_d.ap()[:, eh * 512:(eh + 1) * 512].rearrange(
                        "(o p) e -> p o e", p=128
                    ),
                )
        bo_bc = wbig.tile([128, E], f32, tag="bo")
        nc.gpsimd.dma_start(
            out=bo_bc, in_=bo_d.ap().unsqueeze(0).broadcast_to([128, E])
        )

        # --- phase P: project own-token K/V (+ all-gather) then Q ---
        with (
            tc.tile_pool(name="stage", bufs=4) as stage,
            tc.tile_pool(name="projps", bufs=3, space="PSUM") as projps,
        ):
            for g in range(2):
                for hbl in range(4):
                    hb = g * 4 + hbl
                    ps = projps.tile([128, NQ], f32, tag="proj")
                    for ec in range(8):
                        nc.tensor.matmul(
                            ps, wk_sb[:, ec, hb * 128:(hb + 1) * 128],
                            xT[:, ec, :],
                            start=(ec == 0), stop=(ec == 7),
                        )
                    kst = stage.tile([128, NQ], f8, tag="kst")
                    nc.scalar.copy(out=kst, in_=ps)
                    nc.scalar.dma_start(
                        out=ag_in[g].ap()[hbl * 65536:(hbl + 1) * 65536].rearrange(
                            "(p t) -> p t", p=128
                        ),
                        in_=kst,
                    )
                for jj in range(4):
                    jsl = slice(jj * 128, (jj + 1) * 128)
                    ps = projps.tile([128, NQ], f32, tag="proj")
                    for ec in range(8):
                        nc.tensor.matmul(
                            ps, xT[:, ec, jsl],
                            wv_sb[:, ec, g * 512:(g + 1) * 512],
                            start=(ec == 0), stop=(ec == 7),
                        )
                    psv = ps.rearrange("p (l c) -> p l c", l=4)
                    vst = stage.tile([128, 4, 192], bf16, tag="vst")
                    nc.scalar.copy(out=vst[:, :, 0:64], in_=psv[:, :, 0:64])
                    nc.scalar.copy(out=vst[:, :, 128:192], in_=psv[:, :, 64:128])
                    nc.scalar.activation(
                        out=vst[:, :, 64:128], in_=psv[:, :, 0:64],
                        func=mybir.ActivationFunctionType.Copy,
                        scale=0.0, bias=1.0,
                    )
                    for hbl in range(4):
                        off = VOFF + hbl * 196608 + jj * 49152
                        nc.scalar.dma_start(
                            out=ag_in[g].ap()[off:off + 49152].rearrange(
                                "(p c) -> p c", p=128
                            ),
                            in_=vst[:, hbl, :].bitcast(f8),
                        )
                nc.gpsimd.collective_compute(
                    "AllGather",
                    mybir.AluOpType.bypass,
                    replica_groups=[list(g_) for g_ in GROUPS],
                    ins=[ag_in[g].ap().opt()],
                    outs=[ag_out[g].ap().opt()],
                )

            for hb in range(HB):
                ps = projps.tile([128, NQ], f32, tag="proj")
                for ec in range(8):
                    nc.tensor.matmul(
                        ps, wq_sb[:, ec, hb * 128:(hb + 1) * 128],
                        xT[:, ec, :],
                        start=(ec == 0), stop=(ec == 7),
                    )
                nc.vector.tensor_scalar_add(
                    out=qT[:, hb, :], in0=ps, scalar1=bq_sb[:, hb:hb + 1]
                )

        # --- phase A: attention per head-block over gathered K/V ---
        with (
            tc.tile_pool(name="kpool", bufs=2) as kpool,
            tc.tile_pool(name="vpool", bufs=2) as vpool,
            tc.tile_pool(name="expool", bufs=3) as expool,
            tc.tile_pool(name="dvp", bufs=2) as dvp,
            tc.tile_pool(name="stps", bufs=3, space="PSUM") as stps,
            tc.tile_pool(name="ctxps", bufs=2, space="PSUM") as ctxps,
        ):
            for hb in range(HB):
                g, hbl = hb // 4, hb % 4

                kT = kpool.tile([128, S], f8, tag="kT")
                for r in range(4):
                    nc.sync.dma_start(
                        out=kT[:, r * 512:(r + 1) * 512],
                        in_=ag_out[g].ap()[
                            r, hbl * 65536:(hbl + 1) * 65536
                        ].rearrange("(p t) -> p t", p=128),
                    )
                # von[:, j, :]: [h0_v(64) | ones(64) | h1_v(64)] per key tile;
                # one fully-contiguous load per source rank.
                von = vpool.tile([128, NJ, 192], bf16, tag="von")
                for r in range(4):
                    off = VOFF + hbl * 196608
                    nc.sync.dma_start(
                        out=von[:, r * 4:(r + 1) * 4, :].bitcast(f8),
                        in_=ag_out[g].ap()[r, off:off + 196608].rearrange(
                            "(jj p c) -> p jj c", p=128, c=384
                        ),
                    )

                # attention, h0/h1 interleaved per key tile: the two score
                # matmuls use disjoint PE row-halves. One exp op covers both
                # heads' scores; DVE_J tiles fast-exp on DVE, rest on ACT.
                ctx0 = ctxps.tile([128, NQ], f32, tag="ctx")
                ctx1 = ctxps.tile([128, NQ], f32, tag="ctx")
                ctx_ps = [ctx0, ctx1]
                for j in range(NJ):
                    jsl = slice(j * 128, (j + 1) * 128)
                    st2 = stps.tile([128, 2, NQ], f32, tag="st")
                    for h in range(2):
                        hr = slice(64 * h, 64 * h + 64)
                        nc.tensor.matmul(
                            st2[:, h, :], kT[hr, jsl], qT[hr, hb, :],
                            start=True, stop=True,
                        )
                    if j in DVE_J:
                        exi = expool.tile([128, 2 * NQ], i16, tag="exi")
                        nc.vector.tensor_scalar(
                            out=exi,
                            in0=st2.rearrange("p a b -> p (a b)"),
                            scalar1=EXP_SA, scalar2=EXP_SB,
                            op0=mybir.AluOpType.mult, op1=mybir.AluOpType.add,
                        )
                        exv = exi.bitcast(bf16)
                    else:
                        ex2 = expool.tile([128, 2 * NQ], bf16, tag="ex")
                        nc.scalar.activation(
                            out=ex2,
                            in_=st2.rearrange("p a b -> p (a b)"),
                            func=mybir.ActivationFunctionType.Exp, scale=0.125,
                        )
                        exv = ex2
                    # h0 lhsT = [v|ones] -> denominators on parts 64:128;
                    # h1 lhsT = [ones|v] -> denominators on parts 0:64.
                    nc.tensor.matmul(
                        ctx0, von[:, j, 0:128], exv[:, 0:NQ],
                        start=(j == 0), stop=(j == NJ - 1),
                    )
                    nc.tensor.matmul(
                        ctx1, von[:, j, 64:192], exv[:, NQ:2 * NQ],
                        start=(j == 0), stop=(j == NJ - 1),
                    )

                # normalization: gather both heads' denominator copies into
                # one SBUF tile (h0 rows 64:128, h1 rows 0:64), PE-broadcast
                # them to the opposite half via cb64, one reciprocal over all
                # 128 partitions, then scale each head's numerators.
                l_sb = dvp.tile([128, NQ], f32r, tag="lsb")
                nc.vector.tensor_copy(out=l_sb[64:128, :], in_=ctx0[64:128, :])
                nc.vector.tensor_copy(out=l_sb[0:64, :], in_=ctx1[0:64, :])
                lr_ps = stps.tile([128, NQ], f32, tag="st")
                nc.tensor.matmul(lr_ps, cbX, l_sb, start=True, stop=True)
                recip = dvp.tile([128, NQ], f32, tag="recip")
                nc.vector.reciprocal_approx_fast(out=recip, in_=lr_ps)
                cn = dvp.tile([128, NQ], bf16, tag="cn")
                nc.vector.tensor_mul(
                    out=cn[0:64, :], in0=ctx0[0:64, :], in1=recip[0:64, :]
                )
                nc.vector.tensor_mul(
                    out=cn[64:128, :], in0=ctx1[64:128, :], in1=recip[64:128, :]
                )
                pair_ps = stps.tile([128, NQ], f32, tag="st")
                nc.tensor.matmul(
                    pair_ps, stk2[0:64, 0, :], cn[0:64, :],
                    start=True, stop=False,
                )
                nc.tensor.matmul(
                    pair_ps, stk2[64:128, 1, :], cn[64:128, :],
                    start=False, stop=True,
                )
                nc.vector.tensor_copy(out=ctxT_sb[:, hb, :], in_=pair_ps)

        # --- phase C: output projection for own 512-token slice ---
        with (
            tc.tile_pool(name="ph4", bufs=3) as ph4,
            tc.tile_pool(name="ph4ps", bufs=2, space="PSUM") as ph4ps,
        ):
            for tt in range(4):
                for eh in range(2):
                    esl = slice(eh * 512, (eh + 1) * 512)
                    ps_o = ph4ps.tile([128, 512], f32, tag="o")
                    for hb in range(HB):
                        nc.tensor.matmul(
                            ps_o,
                            ctxT_sb[:, hb, tt * 128:(tt + 1) * 128],
                            wo_sb[:, hb, esl],
                            start=(hb == 0), stop=(hb == HB - 1),
                        )
                    o_sb = ph4.tile([128, 512], f32, tag="osb")
                    nc.vector.tensor_add(out=o_sb, in0=ps_o, in1=bo_bc[:, esl])
                    nc.sync.dma_start(
                        out=out_d.ap()[tt * 128:(tt + 1) * 128, esl], in_=o_sb
                    )

    nc.finalize()
    return nc


def _get_nc():
    if "nc" not in _CACHE:
        _CACHE["nc"] = _build()
    return _CACHE["nc"]


def kernel(x, Wq, bq, Wk, bk, Wv, bv, Wo, bo, **_ignored):
    import ml_dtypes
    from concourse.bass_utils import run_bass_kernel_spmd

    bf = ml_dtypes.bfloat16
    x = np.asarray(x, dtype=np.float32)
    Wqb = np.ascontiguousarray(np.asarray(Wq, dtype=np.float32).astype(bf))
    Wkb = np.ascontiguousarray(np.asarray(Wk, dtype=np.float32).astype(bf))
    Wvb = np.ascontiguousarray(np.asarray(Wv, dtype=np.float32).astype(bf))
    Wob = np.ascontiguousarray(np.asarray(Wo, dtype=np.float32).astype(bf))
    bq = np.ascontiguousarray(np.asarray(bq, dtype=np.float32))
    # K bias is softmax-invariant; V bias folds into the output bias exactly.
    bo2 = np.ascontiguousarray(
        (
            np.asarray(bo, dtype=np.float64)
            + np.asarray(bv, dtype=np.float64) @ np.asarray(Wo, dtype=np.float64)
        ).astype(np.float32)
    )

    in_maps = []
    for c in range(NCORES):
        b, q0 = c // 4, (c % 4) * NQ
        xt = np.ascontiguousarray(x[b, q0:q0 + NQ].T.astype(bf))
        in_maps.append(
            {
                "xt": xt,
                "wq": Wqb, "wk": Wkb, "wv": Wvb, "wo": Wob,
                "bq": bq, "bo": bo2,
            }
        )

    nc = _get_nc()
    import os

    trace = bool(int(os.environ.get("MHA_TRACE", "0")))
    res = run_bass_kernel_spmd(
        nc, in_maps, core_ids=list(range(NCORES)), trace=trace
    )
    if trace:
        _CACHE["last_results"] = res
    _CACHE["res"] = res
    out = np.empty((B, S, E), dtype=np.float32)
    for c in range(NCORES):
        b, q0 = c // 4, (c % 4) * NQ
        out[b, q0:q0 + NQ] = res.results[c]["out"]
    return out


# revision 19
# speedup vs baseline: 1.1477x; 1.0788x over previous
"""Multi-head attention (B=2, S=2048, E=1024, H=16) on 8 Trainium2 NeuronCores.

Sharding v3 — query-block data parallel with an early K/V all-gather:
core c owns batch b=c//4 and query tokens q0=(c%4)*512. Each core projects
Q/K/V only for its OWN 512 tokens (1/4 of the K/V projection work of the
zero-collective scheme), the 4 cores sharing a batch all-gather K and V
through internal DRAM (one collective per head-block half so the first
overlaps the second half of the projections), and each core runs attention
over all 2048 keys for all 16 heads plus the output projection for its own
512 tokens.

Collective payload engineering:
 - K and Q are quantized to fp8-e3m4 (the score matmul runs fp8xfp8 at the
   same 1 cycle/row as bf16; the ~1.5% element error on scores washes out in
   the softmax average). That halves the K bytes through the collective.
 - V ships in "plane" layout [h0_v(64) | ones(64) | h1_v(64)] per key tile,
   so each attention-side load is one fully contiguous 128x768B DMA and the
   ones columns for the PE row-sum (softmax denominator) trick arrive
   in-band: ctx lhsT for h0 is cols 0:128, for h1 cols 64:192 (denominators
   land mirrored on partitions 0:64 for h1; normalization accounts for it).

Softmax-exp is the scalar-engine bottleneck (16.8M exps/core), so DVE_J key
tiles per head-block run on the vector engine instead via a bf16 Schraudolph
fast-exp (int16 bit-trick, ~3% per-element, washes out in the softmax
average), keeping ACT, DVE and PE all busy (~11us/head-block each).

Bias algebra: the K bias is softmax-invariant (q.bk is constant across keys)
and the V bias is a constant vector added to every context row, so both are
dropped on-device; the V bias is folded on the host into the output bias
(bo2 = bo + bv @ Wo). Only the Q bias is applied on-chip.
"""

import sys

if "/opt/trn_rl_repo" not in sys.path:
    sys.path.insert(0, "/opt/trn_rl_repo")

import numpy as np

B, S, E, H, DH = 2, 2048, 1024, 16, 64
NCORES = 8
NQ = 512           # query tokens per core (also its owned K/V token block)
HB = 8             # head-blocks of 128 v-dims (2 heads) each
NJ = 16            # 128-wide key tiles over the full 2048 keys
GROUPS = ((0, 1, 2, 3), (4, 5, 6, 7))

# all-gather buffer layout in fp8 elements (= bytes), per group g:
#   K part: [4 hbl][128 dh][512 keys] fp8            at hbl*65536
#   V part: [4 hbl][128 tok][4 jj][384 (=192 bf16)]  at VOFF + hbl*196608
# (tok-major within a head-block so the attention-side load of one source
#  rank's V for one head-block is a single fully-contiguous 1536B-per-row DMA)
VOFF = 4 * 128 * 512
AGIN = VOFF + 4 * 4 * 128 * 384

# vector-engine fast-exp (int16 bits -> bf16): e^(0.125*s) for raw score s
EXP_SA = 23.083120654223414
EXP_SB = 16250.65
# key tiles whose exp runs on the vector engine (rest on scalar engine)
DVE_J = (4, 6, 8, 10, 12, 14)

_CACHE = {}


def _enable_ldw_opt():
    """The stock walrus invocation passes --enable-ldw-opt=false; LDWEIGHTS
    then serializes with every matmul (~107ns tax per weight swap). Flip it
    to true for this kernel's compilation only."""
    import os

    if not bool(int(os.environ.get("MHA_LDWOPT", "0"))):
        # --enable-ldw-opt=true crashes walrus codegen (visitInstLdweights),
        # so the LDWEIGHTS-per-matmul tax stays.
        return
    import concourse.bass_utils as bu

    if getattr(bu.run_command, "_mha_ldw", False):
        return
    orig = bu.run_command

    def patched(argv, **kwargs):
        argv = [
            a.replace("--enable-ldw-opt=false", "--enable-ldw-opt=true")
            if isinstance(a, str) else a
            for a in argv
        ]
        return orig(argv, **kwargs)

    patched._mha_ldw = True
    bu.run_command = patched


def _build():
    from contextlib import ExitStack

    import concourse.bacc as bacc
    import concourse.mybir as mybir
    import concourse.tile as tile
    from concourse.masks import make_identity

    _enable_ldw_opt()

    f32 = mybir.dt.float32
    f32r = mybir.dt.float32r
    bf16 = mybir.dt.bfloat16
    f8 = mybir.dt.float8e3
    i16 = mybir.dt.int16

    nc = bacc.Bacc("TRN2", num_devices=NCORES)

    xt_d = nc.declare_dram_parameter("xt", [E, NQ], bf16, isOutput=False)
    wq_d = nc.declare_dram_parameter("wq", [E, E], bf16, isOutput=False)
    wk_d = nc.declare_dram_parameter("wk", [E, E], bf16, isOutput=False)
    wv_d = nc.declare_dram_parameter("wv", [E, E], bf16, isOutput=False)
    wo_d = nc.declare_dram_parameter("wo", [E, E], bf16, isOutput=False)
    bq_d = nc.declare_dram_parameter("bq", [E], f32, isOutput=False)
    bo_d = nc.declare_dram_parameter("bo", [E], f32, isOutput=False)
    out_d = nc.declare_dram_parameter("out", [NQ, E], f32, isOutput=True)

    ag_in = [nc.dram_tensor(f"agin{g}", [AGIN], f8) for g in range(2)]
    ag_out = [nc.dram_tensor(f"agout{g}", [4, AGIN], f8) for g in range(2)]

    with tile.TileContext(nc) as tc, ExitStack() as ctx:
        singles = ctx.enter_context(tc.tile_pool(name="singles", bufs=1))

        # wk first on the sync queue: the very first matmul needs only
        # wk chunk 0 + xT chunk 0, so keep the gating transfers small.
        wk_sb = singles.tile([128, 8, E], bf16, tag="wk")
        xT = singles.tile([128, 8, NQ], bf16, tag="xT")
        for oc in range(4):
            nc.sync.dma_start(
                out=wk_sb[:, 2 * oc:2 * oc + 2, :],
                in_=wk_d.ap()[oc * 256:(oc + 1) * 256, :].rearrange(
                    "(o p) c -> p o c", p=128
                ),
            )
            for e2 in range(2):
                o = 2 * oc + e2
                nc.sync.dma_start(
                    out=xT[:, o, :],
                    in_=xt_d.ap()[o * 128:(o + 1) * 128, :].rearrange(
                        "(o p) t -> p o t", p=128
                    ),
                )
        wq_sb = singles.tile([128, 8, E], bf16, tag="wq")
        for oc in range(4):
            nc.sync.dma_start(
                out=wq_sb[:, 2 * oc:2 * oc + 2, :],
                in_=wq_d.ap()[oc * 256:(oc + 1) * 256, :].rearrange(
                    "(o p) c -> p o c", p=128
                ),
            )

        ident = singles.tile([128, 128], f32)
        make_identity(nc, ident)

        # cbX: block-antidiagonal 1/64 — one matmul averages the 64 identical
        # denominator copies of each head AND swaps them to the opposite
        # partition half (h0 denominators live on parts 64:128, its
        # numerators on 0:64; h1 is mirrored).
        cbX = singles.tile([128, 128], f32r, tag="cbX")
        nc.vector.tensor_scalar(
            out=cbX, in0=ident, scalar1=0.0, scalar2=0.0,
            op0=mybir.AluOpType.mult, op1=mybir.AluOpType.add,
        )
        for rs, cs in ((slice(64, 128), slice(0, 64)), (slice(0, 64), slice(64, 128))):
            nc.vector.tensor_scalar(
                out=cbX[rs, cs], in0=ident[rs, cs], scalar1=0.0, scalar2=1.0 / 64.0,
                op0=mybir.AluOpType.mult, op1=mybir.AluOpType.add,
            )
        # stk2: pair-stacking weights. h0: I64 at parts 0:64 -> cols 0:64;
        # h1: I64 at parts 64:128 -> cols 64:128 (h1 numerators live on the
        # upper partition half because its von plane is [ones | v]).
        stk2 = singles.tile([128, 2, 128], bf16, tag="stk2")
        for h in range(2):
            nc.vector.tensor_scalar(
                out=stk2[:, h, :], in0=ident,
                scalar1=0.0, scalar2=0.0,
                op0=mybir.AluOpType.mult, op1=mybir.AluOpType.add,
            )
        nc.vector.tensor_copy(out=stk2[0:64, 0, 0:64], in_=ident[0:64, 0:64])
        nc.vector.tensor_copy(out=stk2[64:128, 1, 64:128], in_=ident[64:128, 64:128])

        # Q bias: [128, 8] (col = head-block)
        bq_sb = singles.tile([128, 8], f32, tag="bq")
        nc.sync.dma_start(
            out=bq_sb, in_=bq_d.ap().rearrange("(o p) -> p o", p=128)
        )

        qT = singles.tile([128, 8, NQ], f8, tag="qT")       # own q^T, fp8
        ctxT_sb = singles.tile([128, HB, NQ], bf16, tag="ctxT")

        # wv resident for the natural-layout V projection; wo for phase C.
        wbig = ctx.enter_context(tc.tile_pool(name="wbig", bufs=1))
        wv_sb = wbig.tile([128, 8, E], bf16, tag="wv")
        wo_sb = wbig.tile([128, 8, E], bf16, tag="wo")
        for eh in range(2):
            nc.gpsimd.dma_start(
                out=wv_sb[:, :, eh * 512:(eh + 1) * 512],
                in_=wv_d.ap()[:, eh * 512:(eh + 1) * 512].rearrange(
                    "(o p) e -> p o e", p=128
                ),
            )
        bo_bc = wbig.tile([128, E], f32, tag="bo")
        nc.gpsimd.dma_start(
            out=bo_bc, in_=bo_d.ap().unsqueeze(0).broadcast_to([128, E])
        )

        # --- phase P: project own-token K/V (+ all-gather) then Q ---
        with (
            tc.tile_pool(name="stage", bufs=4) as stage,
            tc.tile_pool(name="projps", bufs=3, space="PSUM") as projps,
        ):
            for g in range(2):
                for hbl in range(4):
                    hb = g * 4 + hbl
                    ps = projps.tile([128, NQ], f32, tag="proj")
                    for ec in range(8):
                        nc.tensor.matmul(
                            ps, wk_sb[:, ec, hb * 128:(hb + 1) * 128],
                            xT[:, ec, :],
                            start=(ec == 0), stop=(ec == 7),
                        )
                    kst = stage.tile([128, NQ], f8, tag="kst")
                    nc.scalar.copy(out=kst, in_=ps)
                    nc.scalar.dma_start(
                        out=ag_in[g].ap()[hbl * 65536:(hbl + 1) * 65536].rearrange(
                            "(p t) -> p t", p=128
                        ),
                        in_=kst,
                    )
                for jj in range(4):
                    jsl = slice(jj * 128, (jj + 1) * 128)
                    ps = projps.tile([128, NQ], f32, tag="proj")
                    for ec in range(8):
                        nc.tensor.matmul(
                            ps, xT[:, ec, jsl],
                            wv_sb[:, ec, g * 512:(g + 1) * 512],
                            start=(ec == 0), stop=(ec == 7),
                        )
                    psv = ps.rearrange("p (l c) -> p l c", l=4)
                    vst = stage.tile([128, 4, 192], bf16, tag="vst")
                    nc.scalar.copy(out=vst[:, :, 0:64], in_=psv[:, :, 0:64])
                    nc.scalar.copy(out=vst[:, :, 128:192], in_=psv[:, :, 64:128])
                    nc.scalar.activation(
                        out=vst[:, :, 64:128], in_=psv[:, :, 0:64],
                        func=mybir.ActivationFunctionType.Copy,
                        scale=0.0, bias=1.0,
                    )
                    for hbl in range(4):
                        off = VOFF + hbl * 196608
                        nc.scalar.dma_start(
                            out=ag_in[g].ap()[off:off + 196608].rearrange(
                                "(p jj c) -> p jj c", p=128, c=384
                            )[:, jj, :],
                            in_=vst[:, hbl, :].bitcast(f8),
                        )
                nc.gpsimd.collective_compute(
                    "AllGather",
                    mybir.AluOpType.bypass,
                    replica_groups=[list(g_) for g_ in GROUPS],
                    ins=[ag_in[g].ap().opt()],
                    outs=[ag_out[g].ap().opt()],
                )

            for hb in range(HB):
                ps = projps.tile([128, NQ], f32, tag="proj")
                for ec in range(8):
                    nc.tensor.matmul(
                        ps, wq_sb[:, ec, hb * 128:(hb + 1) * 128],
                        xT[:, ec, :],
                        start=(ec == 0), stop=(ec == 7),
                    )
                nc.vector.tensor_scalar_add(
                    out=qT[:, hb, :], in0=ps, scalar1=bq_sb[:, hb:hb + 1]
                )

        # --- phase A: attention per head-block over gathered K/V ---
        with (
            tc.tile_pool(name="kpool", bufs=2) as kpool,
            tc.tile_pool(name="vpool", bufs=2) as vpool,
            tc.tile_pool(name="expool", bufs=3) as expool,
            tc.tile_pool(name="dvp", bufs=2) as dvp,
            tc.tile_pool(name="stps", bufs=2, space="PSUM") as stps,
            tc.tile_pool(name="ctxps", bufs=4, space="PSUM") as ctxps,
        ):
            # wo is only needed by phase C; loading it here keeps the early
            # HBM bandwidth for the projection inputs.
            for eh in range(2):
                nc.sync.dma_start(
                    out=wo_sb[:, :, eh * 512:(eh + 1) * 512],
                    in_=wo_d.ap()[:, eh * 512:(eh + 1) * 512].rearrange(
                        "(o p) e -> p o e", p=128
                    ),
                )

            def emit_norm(hb, ctx0, ctx1, recip=None):
                """Normalization for a finished head-block, split in two so
                its matmuls interleave behind the next head-block's scores
                (keeps the PE stream dense; see caller)."""
                if recip is None:
                    l_sb = dvp.tile([128, NQ], f32r, tag="lsb")
                    nc.vector.tensor_copy(
                        out=l_sb[64:128, :], in_=ctx0[64:128, :]
                    )
                    nc.vector.tensor_copy(out=l_sb[0:64, :], in_=ctx1[0:64, :])
                    lr_ps = stps.tile([128, NQ], f32, tag="st")
                    nc.tensor.matmul(lr_ps, cbX, l_sb, start=True, stop=True)
                    recip = dvp.tile([128, NQ], f32, tag="recip")
                    nc.vector.reciprocal_approx_fast(out=recip, in_=lr_ps)
                    return recip
                cn = dvp.tile([128, NQ], bf16, tag="cn")
                nc.vector.tensor_mul(
                    out=cn[0:64, :], in0=ctx0[0:64, :], in1=recip[0:64, :]
                )
                nc.vector.tensor_mul(
                    out=cn[64:128, :], in0=ctx1[64:128, :], in1=recip[64:128, :]
                )
                pair_ps = stps.tile([128, NQ], f32, tag="st")
                nc.tensor.matmul(
                    pair_ps, stk2[0:64, 0, :], cn[0:64, :],
                    start=True, stop=False,
                )
                nc.tensor.matmul(
                    pair_ps, stk2[64:128, 1, :], cn[64:128, :],
                    start=False, stop=True,
                )
                nc.vector.tensor_copy(out=ctxT_sb[:, hb, :], in_=pair_ps)
                return None

            pending = None  # (hb, ctx0, ctx1) awaiting normalization
            for hb in range(HB):
                g, hbl = hb // 4, hb % 4

                kT = kpool.tile([128, S], f8, tag="kT")
                for r in range(4):
                    nc.sync.dma_start(
                        out=kT[:, r * 512:(r + 1) * 512],
                        in_=ag_out[g].ap()[
                            r, hbl * 65536:(hbl + 1) * 65536
                        ].rearrange("(p t) -> p t", p=128),
                    )
                # von[:, j, :]: [h0_v(64) | ones(64) | h1_v(64)] per key tile;
                # one fully-contiguous load per source rank.
                von = vpool.tile([128, NJ, 192], bf16, tag="von")
                for r in range(4):
                    off = VOFF + hbl * 196608
                    nc.sync.dma_start(
                        out=von[:, r * 4:(r + 1) * 4, :].rearrange(
                            "p a b -> p (a b)"
                        ).bitcast(f8),
                        in_=ag_out[g].ap()[r, off:off + 196608].rearrange(
                            "(p x) -> p x", p=128
                        ),
                    )

                # attention, h0/h1 interleaved per key tile: the two score
                # matmuls use disjoint PE row-halves. One exp op covers both
                # heads' scores; DVE_J tiles fast-exp on DVE, rest on ACT.
                # The ctx matmuls trail the scores by one key tile so the PE
                # never waits on an exp in flight, and the previous
                # head-block's normalization slots into the same stream.
                ctx0 = ctxps.tile([128, NQ], f32, tag="ctx")
                ctx1 = ctxps.tile([128, NQ], f32, tag="ctx")
                exq = [None] * NJ
                recip_prev = None
                for j in range(NJ):
                    jsl = slice(j * 128, (j + 1) * 128)
                    st2 = stps.tile([128, 2, NQ], f32, tag="st")
                    for h in range(2):
                        hr = slice(64 * h, 64 * h + 64)
                        nc.tensor.matmul(
                            st2[:, h, :], kT[hr, jsl], qT[hr, hb, :],
                            start=True, stop=True,
                        )
                    if j in DVE_J:
                        exi = expool.tile([128, 2 * NQ], i16, tag="exi")
                        nc.vector.tensor_scalar(
                            out=exi,
                            in0=st2.rearrange("p a b -> p (a b)"),
                            scalar1=EXP_SA, scalar2=EXP_SB,
                            op0=mybir.AluOpType.mult, op1=mybir.AluOpType.add,
                        )
                        exq[j] = exi.bitcast(bf16)
                    else:
                        ex2 = expool.tile([128, 2 * NQ], bf16, tag="ex")
                        nc.scalar.activation(
                            out=ex2,
                            in_=st2.rearrange("p a b -> p (a b)"),
                            func=mybir.ActivationFunctionType.Exp, scale=0.125,
                        )
                        exq[j] = ex2
                    if j == 1 and pending is not None:
                        recip_prev = emit_norm(*pending)
                    if j == 3 and pending is not None:
                        emit_norm(*pending, recip_prev)
                        pending = None
                    for jc in ([j - 1] if j < NJ - 1 else [j - 1, j]):
                        if jc < 0:
                            continue
                        exv = exq[jc]
                        exq[jc] = None
                        # h0 lhsT = [v|ones] -> denominators on parts 64:128;
                        # h1 lhsT = [ones|v] -> denominators on parts 0:64.
                        nc.tensor.matmul(
                            ctx0, von[:, jc, 0:128], exv[:, 0:NQ],
                            start=(jc == 0), stop=(jc == NJ - 1),
                        )
                        nc.tensor.matmul(
                            ctx1, von[:, jc, 64:192], exv[:, NQ:2 * NQ],
                            start=(jc == 0), stop=(jc == NJ - 1),
                        )
                pending = (hb, ctx0, ctx1)
            recip_prev = emit_norm(*pending)
            emit_norm(*pending, recip_prev)

        # --- phase C: output projection for own 512-token slice ---
        with (
            tc.tile_pool(name="ph4", bufs=3) as ph4,
            tc.tile_pool(name="ph4ps", bufs=2, space="PSUM") as ph4ps,
        ):
            for tt in range(4):
                for eh in range(2):
                    esl = slice(eh * 512, (eh + 1) * 512)
                    ps_o = ph4ps.tile([128, 512], f32, tag="o")
                    for hb in range(HB):
                        nc.tensor.matmul(
                            ps_o,
                            ctxT_sb[:, hb, tt * 128:(tt + 1) * 128],
                            wo_sb[:, hb, esl],
                            start=(hb == 0), stop=(hb == HB - 1),
                        )
                    o_sb = ph4.tile([128, 512], f32, tag="osb")
                    nc.vector.tensor_add(out=o_sb, in0=ps_o, in1=bo_bc[:, esl])
                    nc.sync.dma_start(
                        out=out_d.ap()[tt * 128:(tt + 1) * 128, esl], in_=o_sb
                    )

    nc.finalize()
    return nc


def _get_nc():
    if "nc" not in _CACHE:
        _CACHE["nc"] = _build()
    return _CACHE["nc"]


def kernel(x, Wq, bq, Wk, bk, Wv, bv, Wo, bo, **_ignored):
    import ml_dtypes
    from concourse.bass_utils import run_bass_kernel_spmd

    bf = ml_dtypes.bfloat16
    x = np.asarray(x, dtype=np.float32)
    Wqb = np.ascontiguousarray(np.asarray(Wq, dtype=np.float32).astype(bf))
    Wkb = np.ascontiguousarray(np.asarray(Wk, dtype=np.float32).astype(bf))
    Wvb = np.ascontiguousarray(np.asarray(Wv, dtype=np.float32).astype(bf))
    Wob = np.ascontiguousarray(np.asarray(Wo, dtype=np.float32).astype(bf))
    bq = np.ascontiguousarray(np.asarray(bq, dtype=np.float32))
    # K bias is softmax-invariant; V bias folds into the output bias exactly.
    bo2 = np.ascontiguousarray(
        (
            np.asarray(bo, dtype=np.float64)
            + np.asarray(bv, dtype=np.float64) @ np.asarray(Wo, dtype=np.float64)
        ).astype(np.float32)
    )

    in_maps = []
    for c in range(NCORES):
        b, q0 = c // 4, (c % 4) * NQ
        xt = np.ascontiguousarray(x[b, q0:q0 + NQ].T.astype(bf))
        in_maps.append(
            {
                "xt": xt,
                "wq": Wqb, "wk": Wkb, "wv": Wvb, "wo": Wob,
                "bq": bq, "bo": bo2,
            }
        )

    nc = _get_nc()
    import os

    trace = bool(int(os.environ.get("MHA_TRACE", "0")))
    res = run_bass_kernel_spmd(
        nc, in_maps, core_ids=list(range(NCORES)), trace=trace
    )
    if trace:
        _CACHE["last_results"] = res
    _CACHE["res"] = res
    out = np.empty((B, S, E), dtype=np.float32)
    for c in range(NCORES):
        b, q0 = c // 4, (c % 4) * NQ
        out[b, q0:q0 + NQ] = res.results[c]["out"]
    return out


# revision 21
# speedup vs baseline: 1.1649x; 1.0150x over previous
"""Multi-head attention (B=2, S=2048, E=1024, H=16) on 8 Trainium2 NeuronCores.

Sharding v3 — query-block data parallel with an early K/V all-gather:
core c owns batch b=c//4 and query tokens q0=(c%4)*512. Each core projects
Q/K/V only for its OWN 512 tokens (1/4 of the K/V projection work of the
zero-collective scheme), the 4 cores sharing a batch all-gather K and V
through internal DRAM (one collective per head-block half so the first
overlaps the second half of the projections), and each core runs attention
over all 2048 keys for all 16 heads plus the output projection for its own
512 tokens.

Collective payload engineering:
 - K and Q are quantized to fp8-e3m4 (the score matmul runs fp8xfp8 at the
   same 1 cycle/row as bf16; the ~1.5% element error on scores washes out in
   the softmax average). That halves the K bytes through the collective.
 - V ships in "plane" layout [h0_v(64) | ones(64) | h1_v(64)] per key tile,
   so each attention-side load is one fully contiguous 128x768B DMA and the
   ones columns for the PE row-sum (softmax denominator) trick arrive
   in-band: ctx lhsT for h0 is cols 0:128, for h1 cols 64:192 (denominators
   land mirrored on partitions 0:64 for h1; normalization accounts for it).

Softmax-exp is the scalar-engine bottleneck (16.8M exps/core), so DVE_J key
tiles per head-block run on the vector engine instead via a bf16 Schraudolph
fast-exp (int16 bit-trick, ~3% per-element, washes out in the softmax
average), keeping ACT, DVE and PE all busy (~11us/head-block each).

Bias algebra: the K bias is softmax-invariant (q.bk is constant across keys)
and the V bias is a constant vector added to every context row, so both are
dropped on-device; the V bias is folded on the host into the output bias
(bo2 = bo + bv @ Wo). Only the Q bias is applied on-chip.
"""

import sys

if "/opt/trn_rl_repo" not in sys.path:
    sys.path.insert(0, "/opt/trn_rl_repo")

import numpy as np

B, S, E, H, DH = 2, 2048, 1024, 16, 64
NCORES = 8
NQ = 512           # query tokens per core (also its owned K/V token block)
HB = 8             # head-blocks of 128 v-dims (2 heads) each
NJ = 16            # 128-wide key tiles over the full 2048 keys
GROUPS = ((0, 1, 2, 3), (4, 5, 6, 7))

# all-gather buffer layout in fp8 elements (= bytes), per group g:
#   K part: [4 hbl][128 dh][512 keys] fp8            at hbl*65536
#   V part: [4 hbl][128 tok][4 jj][384 (=192 bf16)]  at VOFF + hbl*196608
# (tok-major within a head-block so the attention-side load of one source
#  rank's V for one head-block is a single fully-contiguous 1536B-per-row DMA)
VOFF = 4 * 128 * 512
AGIN = VOFF + 4 * 4 * 128 * 384

# vector-engine fast-exp (int16 bits -> bf16): e^(0.125*s) for raw score s
EXP_SA = 23.083120654223414
EXP_SB = 16250.65
# key tiles whose exp runs on the vector engine (rest on scalar engine)
DVE_J = (4, 6, 8, 10, 12, 14)

_CACHE = {}


def _enable_ldw_opt():
    """The stock walrus invocation passes --enable-ldw-opt=false; LDWEIGHTS
    then serializes with every matmul (~107ns tax per weight swap). Flip it
    to true for this kernel's compilation only."""
    import os

    if not bool(int(os.environ.get("MHA_LDWOPT", "0"))):
        # --enable-ldw-opt=true crashes walrus codegen (visitInstLdweights),
        # so the LDWEIGHTS-per-matmul tax stays.
        return
    import concourse.bass_utils as bu

    if getattr(bu.run_command, "_mha_ldw", False):
        return
    orig = bu.run_command

    def patched(argv, **kwargs):
        argv = [
            a.replace("--enable-ldw-opt=false", "--enable-ldw-opt=true")
            if isinstance(a, str) else a
            for a in argv
        ]
        return orig(argv, **kwargs)

    patched._mha_ldw = True
    bu.run_command = patched


def _build():
    from contextlib import ExitStack

    import concourse.bacc as bacc
    import concourse.mybir as mybir
    import concourse.tile as tile
    from concourse.masks import make_identity

    _enable_ldw_opt()

    f32 = mybir.dt.float32
    f32r = mybir.dt.float32r
    bf16 = mybir.dt.bfloat16
    f8 = mybir.dt.float8e3
    i16 = mybir.dt.int16

    nc = bacc.Bacc("TRN2", num_devices=NCORES)

    xt_d = nc.declare_dram_parameter("xt", [E, NQ], bf16, isOutput=False)
    wq_d = nc.declare_dram_parameter("wq", [E, E], bf16, isOutput=False)
    wk_d = nc.declare_dram_parameter("wk", [E, E], bf16, isOutput=False)
    wv_d = nc.declare_dram_parameter("wv", [E, E], bf16, isOutput=False)
    wo_d = nc.declare_dram_parameter("wo", [E, E], bf16, isOutput=False)
    bq_d = nc.declare_dram_parameter("bq", [E], f32, isOutput=False)
    bo_d = nc.declare_dram_parameter("bo", [E], f32, isOutput=False)
    out_d = nc.declare_dram_parameter("out", [NQ, E], f32, isOutput=True)

    ag_in = [nc.dram_tensor(f"agin{g}", [AGIN], f8) for g in range(2)]
    ag_out = [nc.dram_tensor(f"agout{g}", [4, AGIN], f8) for g in range(2)]

    with tile.TileContext(nc) as tc, ExitStack() as ctx:
        singles = ctx.enter_context(tc.tile_pool(name="singles", bufs=1))

        # wk first on the sync queue: the very first matmul needs only
        # wk chunk 0 + xT chunk 0, so keep the gating transfers small.
        wk_sb = singles.tile([128, 8, E], bf16, tag="wk")
        xT = singles.tile([128, 8, NQ], bf16, tag="xT")
        for oc in range(4):
            nc.sync.dma_start(
                out=wk_sb[:, 2 * oc:2 * oc + 2, :],
                in_=wk_d.ap()[oc * 256:(oc + 1) * 256, :].rearrange(
                    "(o p) c -> p o c", p=128
                ),
            )
            for e2 in range(2):
                o = 2 * oc + e2
                nc.sync.dma_start(
                    out=xT[:, o, :],
                    in_=xt_d.ap()[o * 128:(o + 1) * 128, :].rearrange(
                        "(o p) t -> p o t", p=128
                    ),
                )
        wq_sb = singles.tile([128, 8, E], bf16, tag="wq")
        for oc in range(4):
            nc.sync.dma_start(
                out=wq_sb[:, 2 * oc:2 * oc + 2, :],
                in_=wq_d.ap()[oc * 256:(oc + 1) * 256, :].rearrange(
                    "(o p) c -> p o c", p=128
                ),
            )

        ident = singles.tile([128, 128], f32)
        make_identity(nc, ident)

        # cbX: block-antidiagonal 1/64 — one matmul averages the 64 identical
        # denominator copies of each head AND swaps them to the opposite
        # partition half (h0 denominators live on parts 64:128, its
        # numerators on 0:64; h1 is mirrored).
        cbX = singles.tile([128, 128], f32r, tag="cbX")
        nc.vector.tensor_scalar(
            out=cbX, in0=ident, scalar1=0.0, scalar2=0.0,
            op0=mybir.AluOpType.mult, op1=mybir.AluOpType.add,
        )
        for rs, cs in ((slice(64, 128), slice(0, 64)), (slice(0, 64), slice(64, 128))):
            nc.vector.tensor_scalar(
                out=cbX[rs, cs], in0=ident[rs, cs], scalar1=0.0, scalar2=1.0 / 64.0,
                op0=mybir.AluOpType.mult, op1=mybir.AluOpType.add,
            )
        # stk2: pair-stacking weights. h0: I64 at parts 0:64 -> cols 0:64;
        # h1: I64 at parts 64:128 -> cols 64:128 (h1 numerators live on the
        # upper partition half because its von plane is [ones | v]).
        stk2 = singles.tile([128, 2, 128], bf16, tag="stk2")
        for h in range(2):
            nc.vector.tensor_scalar(
                out=stk2[:, h, :], in0=ident,
                scalar1=0.0, scalar2=0.0,
                op0=mybir.AluOpType.mult, op1=mybir.AluOpType.add,
            )
        nc.vector.tensor_copy(out=stk2[0:64, 0, 0:64], in_=ident[0:64, 0:64])
        nc.vector.tensor_copy(out=stk2[64:128, 1, 64:128], in_=ident[64:128, 64:128])

        # Q bias: [128, 8] (col = head-block)
        bq_sb = singles.tile([128, 8], f32, tag="bq")
        nc.sync.dma_start(
            out=bq_sb, in_=bq_d.ap().rearrange("(o p) -> p o", p=128)
        )

        qT = singles.tile([128, 8, NQ], f8, tag="qT")       # own q^T, fp8
        ctxT_sb = singles.tile([128, HB, NQ], bf16, tag="ctxT")

        # wv resident for the natural-layout V projection; wo for phase C.
        wbig = ctx.enter_context(tc.tile_pool(name="wbig", bufs=1))
        wv_sb = wbig.tile([128, 8, E], bf16, tag="wv")
        wo_sb = wbig.tile([128, 8, E], bf16, tag="wo")
        for eh in range(2):
            nc.gpsimd.dma_start(
                out=wv_sb[:, :, eh * 512:(eh + 1) * 512],
                in_=wv_d.ap()[:, eh * 512:(eh + 1) * 512].rearrange(
                    "(o p) e -> p o e", p=128
                ),
            )
        bo_bc = wbig.tile([128, E], f32, tag="bo")
        nc.gpsimd.dma_start(
            out=bo_bc, in_=bo_d.ap().unsqueeze(0).broadcast_to([128, E])
        )

        # --- phase P: project own-token K/V (+ all-gather) then Q ---
        with (
            tc.tile_pool(name="stage", bufs=4) as stage,
            tc.tile_pool(name="projps", bufs=3, space="PSUM") as projps,
            tc.tile_pool(name="warmps", bufs=1, space="PSUM") as warmps,
        ):
            # HAM warmup: the PE clock-gate opens only after ~3.4us of
            # sustained matmul activity. Burn the initial input-DMA wait on
            # dummy matmuls over the first xT chunk so the real projections
            # start at 2.4GHz instead of 1.2GHz.
            warm = warmps.tile([128, NQ], f32, tag="warm")
            for _ in range(12):
                nc.tensor.matmul(
                    warm, xT[:, 0, 0:128], xT[:, 0, :], start=True, stop=True,
                )
            for g in range(2):
                for hbl in range(4):
                    hb = g * 4 + hbl
                    ps = projps.tile([128, NQ], f32, tag="proj")
                    for ec in range(8):
                        nc.tensor.matmul(
                            ps, wk_sb[:, ec, hb * 128:(hb + 1) * 128],
                            xT[:, ec, :],
                            start=(ec == 0), stop=(ec == 7),
                        )
                    kst = stage.tile([128, NQ], f8, tag="kst")
                    nc.scalar.copy(out=kst, in_=ps)
                    nc.sync.dma_start(
                        out=ag_in[g].ap()[hbl * 65536:(hbl + 1) * 65536].rearrange(
                            "(p t) -> p t", p=128
                        ),
                        in_=kst,
                    )
                for jj in range(4):
                    jsl = slice(jj * 128, (jj + 1) * 128)
                    ps = projps.tile([128, NQ], f32, tag="proj")
                    for ec in range(8):
                        nc.tensor.matmul(
                            ps, xT[:, ec, jsl],
                            wv_sb[:, ec, g * 512:(g + 1) * 512],
                            start=(ec == 0), stop=(ec == 7),
                        )
                    psv = ps.rearrange("p (l c) -> p l c", l=4)
                    vst = stage.tile([128, 4, 192], bf16, tag="vst")
                    nc.scalar.copy(out=vst[:, :, 0:64], in_=psv[:, :, 0:64])
                    nc.scalar.copy(out=vst[:, :, 128:192], in_=psv[:, :, 64:128])
                    nc.scalar.activation(
                        out=vst[:, :, 64:128], in_=psv[:, :, 0:64],
                        func=mybir.ActivationFunctionType.Copy,
                        scale=0.0, bias=1.0,
                    )
                    nc.scalar.dma_start(
                        out=ag_in[g].ap()[VOFF:VOFF + 4 * 196608].rearrange(
                            "(l p jj c) -> p l jj c", l=4, p=128, c=384
                        )[:, :, jj, :],
                        in_=vst.rearrange("p l c -> p (l c)").bitcast(f8).rearrange(
                            "p (l c) -> p l c", l=4
                        ),
                    )
                nc.gpsimd.collective_compute(
                    "AllGather",
                    mybir.AluOpType.bypass,
                    replica_groups=[list(g_) for g_ in GROUPS],
                    ins=[ag_in[g].ap().opt()],
                    outs=[ag_out[g].ap().opt()],
                )

            for hb in range(HB):
                ps = projps.tile([128, NQ], f32, tag="proj")
                for ec in range(8):
                    nc.tensor.matmul(
                        ps, wq_sb[:, ec, hb * 128:(hb + 1) * 128],
                        xT[:, ec, :],
                        start=(ec == 0), stop=(ec == 7),
                    )
                nc.vector.tensor_scalar_add(
                    out=qT[:, hb, :], in0=ps, scalar1=bq_sb[:, hb:hb + 1]
                )

            # HAM keepalive: the first all-gather lands well after the last
            # projection; an idle PE would re-throttle to 1.2GHz and restart
            # attention cold. Bridge the wait with dummy matmuls (~25us).
            for _ in range(72):
                nc.tensor.matmul(
                    warm, xT[:, 0, 0:128], xT[:, 0, :], start=True, stop=True,
                )

        # --- phase A: attention per head-block over gathered K/V ---
        with (
            tc.tile_pool(name="kpool", bufs=2) as kpool,
            tc.tile_pool(name="vpool", bufs=2) as vpool,
            tc.tile_pool(name="expool", bufs=3) as expool,
            tc.tile_pool(name="dvp", bufs=2) as dvp,
            tc.tile_pool(name="stps", bufs=2, space="PSUM") as stps,
            tc.tile_pool(name="ctxps", bufs=4, space="PSUM") as ctxps,
        ):
            # wo is only needed by phase C; loading it here keeps the early
            # HBM bandwidth for the projection inputs.
            for eh in range(2):
                nc.sync.dma_start(
                    out=wo_sb[:, :, eh * 512:(eh + 1) * 512],
                    in_=wo_d.ap()[:, eh * 512:(eh + 1) * 512].rearrange(
                        "(o p) e -> p o e", p=128
                    ),
                )

            def emit_norm(hb, ctx0, ctx1, recip=None):
                """Normalization for a finished head-block, split in two so
                its matmuls interleave behind the next head-block's scores
                (keeps the PE stream dense; see caller)."""
                if recip is None:
                    l_sb = dvp.tile([128, NQ], f32r, tag="lsb")
                    nc.vector.tensor_copy(
                        out=l_sb[64:128, :], in_=ctx0[64:128, :]
                    )
                    nc.vector.tensor_copy(out=l_sb[0:64, :], in_=ctx1[0:64, :])
                    lr_ps = stps.tile([128, NQ], f32, tag="st")
                    nc.tensor.matmul(lr_ps, cbX, l_sb, start=True, stop=True)
                    recip = dvp.tile([128, NQ], f32, tag="recip")
                    nc.vector.reciprocal_approx_fast(out=recip, in_=lr_ps)
                    return recip
                cn = dvp.tile([128, NQ], bf16, tag="cn")
                nc.vector.tensor_mul(
                    out=cn[0:64, :], in0=ctx0[0:64, :], in1=recip[0:64, :]
                )
                nc.vector.tensor_mul(
                    out=cn[64:128, :], in0=ctx1[64:128, :], in1=recip[64:128, :]
                )
                pair_ps = stps.tile([128, NQ], f32, tag="st")
                nc.tensor.matmul(
                    pair_ps, stk2[0:64, 0, :], cn[0:64, :],
                    start=True, stop=False,
                )
                nc.tensor.matmul(
                    pair_ps, stk2[64:128, 1, :], cn[64:128, :],
                    start=False, stop=True,
                )
                nc.vector.tensor_copy(out=ctxT_sb[:, hb, :], in_=pair_ps)
                return None

            pending = None  # (hb, ctx0, ctx1) awaiting normalization
            for hb in range(HB):
                g, hbl = hb // 4, hb % 4

                kT = kpool.tile([128, S], f8, tag="kT")
                for r in range(4):
                    nc.sync.dma_start(
                        out=kT[:, r * 512:(r + 1) * 512],
                        in_=ag_out[g].ap()[
                            r, hbl * 65536:(hbl + 1) * 65536
                        ].rearrange("(p t) -> p t", p=128),
                    )
                # von[:, j, :]: [h0_v(64) | ones(64) | h1_v(64)] per key tile;
                # one fully-contiguous load per source rank.
                von = vpool.tile([128, NJ, 192], bf16, tag="von")
                for r in range(4):
                    off = VOFF + hbl * 196608
                    nc.sync.dma_start(
                        out=von[:, r * 4:(r + 1) * 4, :].rearrange(
                            "p a b -> p (a b)"
                        ).bitcast(f8),
                        in_=ag_out[g].ap()[r, off:off + 196608].rearrange(
                            "(p x) -> p x", p=128
                        ),
                    )

                # attention, h0/h1 interleaved per key tile: the two score
                # matmuls use disjoint PE row-halves. One exp op covers both
                # heads' scores; DVE_J tiles fast-exp on DVE, rest on ACT.
                # The ctx matmuls trail the scores by one key tile so the PE
                # never waits on an exp in flight, and the previous
                # head-block's normalization slots into the same stream.
                ctx0 = ctxps.tile([128, NQ], f32, tag="ctx")
                ctx1 = ctxps.tile([128, NQ], f32, tag="ctx")
                exq = [None] * NJ
                recip_prev = None
                for j in range(NJ):
                    jsl = slice(j * 128, (j + 1) * 128)
                    st2 = stps.tile([128, 2, NQ], f32, tag="st")
                    for h in range(2):
                        hr = slice(64 * h, 64 * h + 64)
                        nc.tensor.matmul(
                            st2[:, h, :], kT[hr, jsl], qT[hr, hb, :],
                            start=True, stop=True,
                        )
                    if j in DVE_J:
                        exi = expool.tile([128, 2 * NQ], i16, tag="exi")
                        nc.vector.tensor_scalar(
                            out=exi,
                            in0=st2.rearrange("p a b -> p (a b)"),
                            scalar1=EXP_SA, scalar2=EXP_SB,
                            op0=mybir.AluOpType.mult, op1=mybir.AluOpType.add,
                        )
                        exq[j] = exi.bitcast(bf16)
                    else:
                        ex2 = expool.tile([128, 2 * NQ], bf16, tag="ex")
                        nc.scalar.activation(
                            out=ex2,
                            in_=st2.rearrange("p a b -> p (a b)"),
                            func=mybir.ActivationFunctionType.Exp, scale=0.125,
                        )
                        exq[j] = ex2
                    if j == 1 and pending is not None:
                        recip_prev = emit_norm(*pending)
                    if j == 3 and pending is not None:
                        emit_norm(*pending, recip_prev)
                        pending = None
                    for jc in ([j - 1] if j < NJ - 1 else [j - 1, j]):
                        if jc < 0:
                            continue
                        exv = exq[jc]
                        exq[jc] = None
                        # h0 lhsT = [v|ones] -> denominators on parts 64:128;
                        # h1 lhsT = [ones|v] -> denominators on parts 0:64.
                        nc.tensor.matmul(
                            ctx0, von[:, jc, 0:128], exv[:, 0:NQ],
                            start=(jc == 0), stop=(jc == NJ - 1),
                        )
                        nc.tensor.matmul(
                            ctx1, von[:, jc, 64:192], exv[:, NQ:2 * NQ],
                            start=(jc == 0), stop=(jc == NJ - 1),
                        )
                pending = (hb, ctx0, ctx1)
            recip_prev = emit_norm(*pending)
            emit_norm(*pending, recip_prev)

        # --- phase C: output projection for own 512-token slice ---
        with (
            tc.tile_pool(name="ph4", bufs=3) as ph4,
            tc.tile_pool(name="ph4ps", bufs=2, space="PSUM") as ph4ps,
        ):
            for tt in range(4):
                for eh in range(2):
                    esl = slice(eh * 512, (eh + 1) * 512)
                    ps_o = ph4ps.tile([128, 512], f32, tag="o")
                    for hb in range(HB):
                        nc.tensor.matmul(
                            ps_o,
                            ctxT_sb[:, hb, tt * 128:(tt + 1) * 128],
                            wo_sb[:, hb, esl],
                            start=(hb == 0), stop=(hb == HB - 1),
                        )
                    o_sb = ph4.tile([128, 512], f32, tag="osb")
                    nc.vector.tensor_add(out=o_sb, in0=ps_o, in1=bo_bc[:, esl])
                    nc.sync.dma_start(
                        out=out_d.ap()[tt * 128:(tt + 1) * 128, esl], in_=o_sb
                    )

    nc.finalize()
    return nc


def _get_nc():
    if "nc" not in _CACHE:
        _CACHE["nc"] = _build()
    return _CACHE["nc"]


def kernel(x, Wq, bq, Wk, bk, Wv, bv, Wo, bo, **_ignored):
    import ml_dtypes
    from concourse.bass_utils import run_bass_kernel_spmd

    bf = ml_dtypes.bfloat16
    x = np.asarray(x, dtype=np.float32)
    Wqb = np.ascontiguousarray(np.asarray(Wq, dtype=np.float32).astype(bf))
    Wkb = np.ascontiguousarray(np.asarray(Wk, dtype=np.float32).astype(bf))
    Wvb = np.ascontiguousarray(np.asarray(Wv, dtype=np.float32).astype(bf))
    Wob = np.ascontiguousarray(np.asarray(Wo, dtype=np.float32).astype(bf))
    bq = np.ascontiguousarray(np.asarray(bq, dtype=np.float32))
    # K bias is softmax-invariant; V bias folds into the output bias exactly.
    bo2 = np.ascontiguousarray(
        (
            np.asarray(bo, dtype=np.float64)
            + np.asarray(bv, dtype=np.float64) @ np.asarray(Wo, dtype=np.float64)
        ).astype(np.float32)
    )

    in_maps = []
    for c in range(NCORES):
        b, q0 = c // 4, (c % 4) * NQ
        xt = np.ascontiguousarray(x[b, q0:q0 + NQ].T.astype(bf))
        in_maps.append(
            {
                "xt": xt,
                "wq": Wqb, "wk": Wkb, "wv": Wvb, "wo": Wob,
                "bq": bq, "bo": bo2,
            }
        )

    nc = _get_nc()
    import os

    trace = bool(int(os.environ.get("MHA_TRACE", "0")))
    res = run_bass_kernel_spmd(
        nc, in_maps, core_ids=list(range(NCORES)), trace=trace
    )
    if trace:
        _CACHE["last_results"] = res
    _CACHE["res"] = res
    out = np.empty((B, S, E), dtype=np.float32)
    for c in range(NCORES):
        b, q0 = c // 4, (c % 4) * NQ
        out[b, q0:q0 + NQ] = res.results[c]["out"]
    return out
